# revision 12
# baseline (speedup 1.0000x reference)
"""Trainium2 Bass kernel for nn_NodeModel (GNN message passing).

Math (see reference):
  mesh_agg = scatter_mean(mesh_edge_attr, mesh_dst, N)
  world_agg = scatter_mean(world_edge_attr, world_dst, N)
  h = relu(concat([x, mesh_agg, world_agg]) @ W1 + b1) @ W2 + b2
  out = x + LayerNorm(h) * gamma + beta

Strategy:
  - Nodes are globally sorted by (mesh_degree, world_degree) and packed into
    784 windows of 128 nodes; windows are dealt round-robin to 8 cores so the
    per-program-slot max degrees match across cores (the per-slot edge-slot
    counts are baked into the single SPMD program).
  - Edges are packed host-side into an ELL-ish layout: window slab
    [lane(node)=partition, feat, slot] in bf16, zero padded.  On device the
    scatter-mean becomes one strided tensor_reduce (slot axis innermost) plus
    a per-node reciprocal-degree scale.
  - MLP runs feature-major on the PE: rhs operands (x^T, magg^T, wagg^T) are
    produced with DMA(xbar) transposes in bf16, weights are bf16, psum fp32.
  - LayerNorm runs node-major after a batched DMA tile-transpose; residual add
    against fp32 x.
  - All 8 cores run the same program on different data; host gathers and
    inverse-permutes the output.
"""

import os
import sys
from functools import lru_cache

import numpy as np

sys.path.insert(0, "/opt/trn_rl_repo")

import ml_dtypes

N_NODES = 100000
N_MESH = 600000
N_WORLD = 300000
D = 128
P = 128
C = 8  # cores
EPS = 1e-5
WPC = -(-N_NODES // (C * P))  # 98 windows per core
NW_TOT = C * WPC  # 784 global windows
NS = NW_TOT * P  # 100352 node slots
NB = 4  # windows per MLP batch

BF16 = ml_dtypes.bfloat16

LAST_STATS = {}


# ----------------------------------------------------------------------------
# Host-side packing
# ----------------------------------------------------------------------------

def _even_ceil(a):
    a = np.maximum(a, 2)
    return ((a + 1) // 2) * 2


def _pack(x, mesh_edge_attr, world_edge_attr, mesh_dst, world_dst):
    """Build per-core device buffers + metadata."""
    mesh_dst = np.asarray(mesh_dst).astype(np.int64)
    world_dst = np.asarray(world_dst).astype(np.int64)

    dm = np.bincount(mesh_dst, minlength=N_NODES)
    dw = np.bincount(world_dst, minlength=N_NODES)

    # node order: sorted by (mesh degree, world degree)
    order = np.lexsort((dw, dm))
    pad = NS - N_NODES

    # slot i (0..NS-1): first `pad` slots are dummy nodes, then sorted nodes.
    # global window g = i // P, lane p = i % P ; core = g % C, prog-slot s = g // C
    ipos = np.empty(N_NODES, dtype=np.int64)
    ipos[order] = pad + np.arange(N_NODES)

    dms = np.zeros(NS, dtype=np.int64)
    dws = np.zeros(NS, dtype=np.int64)
    dms[pad:] = dm[order]
    dws[pad:] = dw[order]

    # per-global-window maxima -> per-prog-slot (max over the 8 cores) counts
    wmax_m = dms.reshape(NW_TOT, P).max(axis=1)
    wmax_w = dws.reshape(NW_TOT, P).max(axis=1)
    Tm = _even_ceil(wmax_m.reshape(WPC, C).max(axis=1))  # [WPC]
    Tw = _even_ceil(wmax_w.reshape(WPC, C).max(axis=1))
    com = np.concatenate([[0], np.cumsum(D * Tm)])  # column offsets, len WPC+1
    cow = np.concatenate([[0], np.cumsum(D * Tw)])
    CDTm = int(com[-1])
    CDTw = int(cow[-1])

    def pack_edges(attr, dst, deg, T, co, CDT):
        M = dst.shape[0]
        buf = np.zeros(C * P * CDT, dtype=BF16)
        perm = np.argsort(dst, kind="stable")
        starts = np.concatenate([[0], np.cumsum(deg)])
        dst_sorted = dst[perm]
        k = np.arange(M, dtype=np.int64) - starts[dst_sorted]
        i = ipos[dst_sorted]
        g = i // P
        p = i % P
        c = g % C
        s = g // C
        T_e = T[s]
        base = c * (P * CDT) + p * CDT + co[s] + k
        attr_b = np.ascontiguousarray(attr).astype(BF16)
        d_ar = np.arange(D, dtype=np.int64)
        CH = 120000
        for lo in range(0, M, CH):
            hi = min(lo + CH, M)
            idx = base[lo:hi, None] + d_ar[None, :] * T_e[lo:hi, None]
            buf[idx] = attr_b[perm[lo:hi]]
        return buf.reshape(C, P, CDT)

    mesh_buf = pack_edges(mesh_edge_attr, mesh_dst, dm, Tm, com, CDTm)
    world_buf = pack_edges(world_edge_attr, world_dst, dw, Tw, cow, CDTw)

    # permuted x per core: [C, WPC*P, D]
    i = pad + np.arange(N_NODES)
    g = i // P
    p = i % P
    c = g % C
    s = g // C
    row = s * P + p

    x = np.ascontiguousarray(x, dtype=np.float32)
    x_buf = np.zeros((C, WPC * P, D), dtype=np.float32)
    x_buf[c, row] = x[order]
    xbf_buf = x_buf.astype(BF16)

    # reciprocal degree [C, P, WPC]  (lane-major so [128, WPC] DMAs directly)
    rm = (1.0 / np.maximum(dms, 1)).astype(np.float32)
    rw = (1.0 / np.maximum(dws, 1)).astype(np.float32)
    ga = np.arange(NS) // P
    pa = np.arange(NS) % P
    rm_buf = np.zeros((C, P, WPC), dtype=np.float32)
    rw_buf = np.zeros((C, P, WPC), dtype=np.float32)
    rm_buf[ga % C, pa, ga // C] = rm
    rw_buf[ga % C, pa, ga // C] = rw

    unperm = (c, row)  # out[order] = result[c, row]
    return dict(
        Tm=Tm, Tw=Tw, com=com, cow=cow, CDTm=CDTm, CDTw=CDTw,
        mesh_buf=mesh_buf, world_buf=world_buf,
        x_buf=x_buf, xbf_buf=xbf_buf, rm_buf=rm_buf, rw_buf=rw_buf,
        order=order, unperm=unperm,
    )


# ----------------------------------------------------------------------------
# Device program
# ----------------------------------------------------------------------------

def _build_program(Tm, Tw, com, cow, CDTm, CDTw, has_beta, wpc=WPC):
    from contextlib import ExitStack
    import concourse.bass as bass
    import concourse.tile as tile
    from concourse import bacc, mybir

    f32 = mybir.dt.float32
    bf16 = mybir.dt.bfloat16
    AF = mybir.ActivationFunctionType
    OP = mybir.AluOpType
    AX = mybir.AxisListType

    nc = bacc.Bacc("TRN2", target_bir_lowering=False, debug=False,
                   enable_asserts=False, num_devices=C)

    mesh_d = nc.dram_tensor("mesh_buf", [P, CDTm], bf16, kind="ExternalInput").ap()
    world_d = nc.dram_tensor("world_buf", [P, CDTw], bf16, kind="ExternalInput").ap()
    x_d = nc.dram_tensor("x_buf", [wpc * P, D], f32, kind="ExternalInput").ap()
    xbf_d = nc.dram_tensor("xbf_buf", [wpc * P, D], bf16, kind="ExternalInput").ap()
    rm_d = nc.dram_tensor("rm_buf", [P, wpc], f32, kind="ExternalInput").ap()
    rw_d = nc.dram_tensor("rw_buf", [P, wpc], f32, kind="ExternalInput").ap()
    w1a_d = nc.dram_tensor("w1a", [D, D], bf16, kind="ExternalInput").ap()
    w1b_d = nc.dram_tensor("w1b", [D, D], bf16, kind="ExternalInput").ap()
    w1c_d = nc.dram_tensor("w1c", [D, D], bf16, kind="ExternalInput").ap()
    w2_d = nc.dram_tensor("w2", [D, D], bf16, kind="ExternalInput").ap()
    b1_d = nc.dram_tensor("b1c", [P, 1], f32, kind="ExternalInput").ap()
    b2_d = nc.dram_tensor("b2c", [P, 1], f32, kind="ExternalInput").ap()
    gb_d = nc.dram_tensor("gamma_bc", [P, NB * D], bf16, kind="ExternalInput").ap()
    if has_beta:
        bb_d = nc.dram_tensor("beta_bc", [P, NB * D], f32, kind="ExternalInput").ap()
    out_d = nc.dram_tensor("out_buf", [wpc * P, D], f32, kind="ExternalOutput").ap()

    with tile.TileContext(nc) as tc, ExitStack() as ctx:
        ctx.enter_context(nc.allow_low_precision(
            reason="bf16 intermediates are intentional; DVE accumulates fp32"))
        const = ctx.enter_context(tc.tile_pool(name="const", bufs=1))
        epool = ctx.enter_context(tc.tile_pool(name="edges", bufs=2))
        xpool = ctx.enter_context(tc.tile_pool(name="xin", bufs=2))
        tpool = ctx.enter_context(tc.tile_pool(name="work", bufs=2))
        spool = ctx.enter_context(tc.tile_pool(name="stats", bufs=2))
        psum = ctx.enter_context(tc.tile_pool(name="psum", bufs=2, space="PSUM"))

        def cload(shape, dt, src, tag):
            t = const.tile(shape, dt, tag=tag)
            nc.sync.dma_start(t[:], src)
            return t

        w1a = cload([D, D], bf16, w1a_d, "w1a")
        w1b = cload([D, D], bf16, w1b_d, "w1b")
        w1c = cload([D, D], bf16, w1c_d, "w1c")
        w2 = cload([D, D], bf16, w2_d, "w2")
        b1 = cload([P, 1], f32, b1_d, "b1")
        b2 = cload([P, 1], f32, b2_d, "b2")
        gb = cload([P, NB * D], bf16, gb_d, "gb")
        if has_beta:
            bbt = cload([P, NB * D], f32, bb_d, "bbt")
        rmt = cload([P, wpc], f32, rm_d, "rmt")
        rwt = cload([P, wpc], f32, rw_d, "rwt")
        epsc = const.tile([P, 1], f32, tag="epsc")
        nc.gpsimd.memset(epsc[:], EPS)

        batches = []
        b0 = 0
        while b0 < wpc:
            batches.append((b0, min(NB, wpc - b0)))
            b0 += NB

        for (s0, nb) in batches:
            sl = slice(s0, s0 + nb)
            mcol0, mcol1 = int(com[s0]), int(com[s0 + nb])
            wcol0, wcol1 = int(cow[s0]), int(cow[s0 + nb])

            # ---- loads (batched) ----
            met = epool.tile([P, mcol1 - mcol0], bf16, tag="mesh")
            nc.sync.dma_start(met[:], mesh_d[:, mcol0:mcol1])
            wet = epool.tile([P, wcol1 - wcol0], bf16, tag="world")
            nc.sync.dma_start(wet[:], world_d[:, wcol0:wcol1])

            xt = xpool.tile([P, nb * D], f32, tag="x")
            nc.gpsimd.dma_start(
                xt[:],
                x_d[s0 * P:(s0 + nb) * P, :].rearrange("(j p) d -> p j d", p=P),
            )
            # x^T for the MLP: full transpose straight from DRAM (bf16)
            xT = tpool.tile([P, nb * D], bf16, tag="xT")
            nc.sync.dma_start(xT[:], xbf_d[s0 * P:(s0 + nb) * P, :], transpose=True)

            # ---- scatter-mean via strided reduce + 1/deg scale ----
            maggm = tpool.tile([P, nb * D], bf16, tag="maggm")
            waggm = tpool.tile([P, nb * D], bf16, tag="waggm")
            for j in range(nb):
                s = s0 + j
                tm, tw = int(Tm[s]), int(Tw[s])
                moff = int(com[s]) - mcol0
                woff = int(cow[s]) - wcol0
                msum = tpool.tile([P, D], bf16, tag="msum")
                nc.vector.tensor_reduce(
                    msum[:],
                    met[:, moff:moff + D * tm].rearrange("p (d t) -> p d t", t=tm),
                    axis=AX.X, op=OP.add,
                )
                nc.vector.tensor_scalar(
                    maggm[:, j * D:(j + 1) * D], msum[:], rmt[:, s:s + 1], None,
                    op0=OP.mult,
                )
                wsum = tpool.tile([P, D], bf16, tag="wsum")
                nc.vector.tensor_reduce(
                    wsum[:],
                    wet[:, woff:woff + D * tw].rearrange("p (d t) -> p d t", t=tw),
                    axis=AX.X, op=OP.add,
                )
                nc.vector.tensor_scalar(
                    waggm[:, j * D:(j + 1) * D], wsum[:], rwt[:, s:s + 1], None,
                    op0=OP.mult,
                )

            # batched per-128-block tile transposes (node-major -> feat-major)
            maggT = tpool.tile([P, nb, D], bf16, tag="maggT")
            nc.scalar.dma_start(maggT[:], maggm[:], transpose=True)
            waggT = tpool.tile([P, nb, D], bf16, tag="waggT")
            nc.scalar.dma_start(waggT[:], waggm[:], transpose=True)

            # ---- MLP (feature-major, rhs free dim = nb*128 nodes) ----
            h1 = psum.tile([P, nb * D], f32, tag="h1")
            mm = maggT[:].rearrange("p j d -> p (j d)")
            wm = waggT[:].rearrange("p j d -> p (j d)")
            nc.tensor.matmul(h1[:], w1a[:], xT[:], start=True, stop=False)
            nc.tensor.matmul(h1[:], w1b[:], mm, start=False, stop=False)
            nc.tensor.matmul(h1[:], w1c[:], wm, start=False, stop=True)
            h1s = tpool.tile([P, nb * D], bf16, tag="h1s")
            nc.scalar.activation(h1s[:], h1[:], AF.Relu, bias=b1[:, 0:1])
            h2 = psum.tile([P, nb * D], f32, tag="h2")
            nc.tensor.matmul(h2[:], w2[:], h1s[:], start=True, stop=True)
            yT = tpool.tile([P, nb * D], bf16, tag="yT")
            nc.scalar.activation(yT[:], h2[:], AF.Identity, bias=b2[:, 0:1])

            # ---- LayerNorm (node-major) ----
            yn = tpool.tile([P, nb, D], bf16, tag="yn")
            nc.scalar.dma_start(yn[:], yT[:], transpose=True)

            mv = spool.tile([P, 2 * nb], f32, tag="mv")
            for j in range(nb):
                st6 = spool.tile([P, 6], f32, tag="st6")
                nc.vector.bn_stats(st6[:], yn[:, j, :])
                nc.vector.bn_aggr(mv[:, 2 * j:2 * j + 2], st6[:])
            # sd = sqrt(var + eps) ; a = 1/sd ; bb = -mu * a
            sd = spool.tile([P, nb], f32, tag="sd")
            nc.scalar.activation(sd[:], mv[:, 1::2], AF.Sqrt, bias=epsc[:, 0:1])
            av = spool.tile([P, nb], f32, tag="av")
            nc.vector.reciprocal(av[:], sd[:])
            ngm = spool.tile([P, nb], f32, tag="ngm")
            nc.vector.tensor_scalar(ngm[:], mv[:, 0::2], -1.0, None, op0=OP.mult)
            bbv = spool.tile([P, nb], f32, tag="bbv")
            nc.vector.tensor_tensor(bbv[:], ngm[:], av[:], op=OP.mult)

            tn = tpool.tile([P, nb * D], bf16, tag="tn")
            for j in range(nb):
                # t = yn * a - mu * a  == (yn - mu) * rsqrt(var+eps)
                nc.scalar.activation(
                    tn[:, j * D:(j + 1) * D], yn[:, j, :], AF.Identity,
                    bias=bbv[:, j:j + 1], scale=av[:, j:j + 1],
                )
            gn = tpool.tile([P, nb * D], bf16, tag="gn")
            nc.vector.tensor_tensor(gn[:], tn[:], gb[:, :nb * D], op=OP.mult)
            on = tpool.tile([P, nb * D], f32, tag="on")
            nc.gpsimd.tensor_tensor(on[:], gn[:], xt[:], op=OP.add)
            if has_beta:
                nc.gpsimd.tensor_tensor(on[:], on[:], bbt[:, :nb * D], op=OP.add)

            nc.sync.dma_start(
                out_d[s0 * P:(s0 + nb) * P, :].rearrange("(j p) d -> p j d", p=P),
                on[:],
            )

    nc.compile()
    return nc


_PROGRAM_CACHE = {}


def _get_program(Tm, Tw, com, cow, CDTm, CDTw, has_beta, wpc=WPC):
    key = (tuple(Tm), tuple(Tw), bool(has_beta), wpc)
    if key not in _PROGRAM_CACHE:
        _PROGRAM_CACHE[key] = _build_program(Tm, Tw, com, cow, CDTm, CDTw,
                                             has_beta, wpc)
    return _PROGRAM_CACHE[key]


# ----------------------------------------------------------------------------
# SPMD runner (PJRT over axon), with optional repeat timing
# ----------------------------------------------------------------------------

_RUNNER_CACHE = {}


def _make_runner(nc):
    import jax
    from jax.sharding import Mesh, PartitionSpec, NamedSharding
    from jax.experimental.shard_map import shard_map
    from concourse import mybir
    from concourse.bass2jax import (_bass_exec_p, install_neuronx_cc_hook,
                                    partition_id_tensor)

    install_neuronx_cc_hook()

    partition_name = (nc.partition_id_tensor.name
                      if nc.partition_id_tensor else None)
    in_names, out_names, out_avals = [], [], []
    for alloc in nc.m.functions[0].allocations:
        if not isinstance(alloc, mybir.MemoryLocationSet):
            continue
        name = alloc.memorylocations[0].name
        if alloc.kind == "ExternalInput":
            if name != partition_name:
                in_names.append(name)
        elif alloc.kind == "ExternalOutput":
            out_names.append(name)
            out_avals.append(jax.core.ShapedArray(
                tuple(alloc.tensor_shape), mybir.dt.np(alloc.dtype)))
    n_params = len(in_names)
    all_names = in_names + out_names
    if partition_name is not None:
        all_names = all_names + [partition_name]

    def _body(*args):
        operands = list(args)
        if partition_name is not None:
            operands.append(partition_id_tensor())
        outs = _bass_exec_p.bind(
            *operands,
            out_avals=tuple(out_avals),
            in_names=tuple(all_names),
            out_names=tuple(out_names),
            lowering_input_output_aliases=(),
            sim_require_finite=True,
            sim_require_nnan=True,
            nc=nc,
        )
        return tuple(outs)

    devices = jax.devices()[:C]
    mesh = Mesh(np.asarray(devices), ("core",))
    spec = PartitionSpec("core")
    n_out = len(out_names)
    fn = jax.jit(
        shard_map(_body, mesh=mesh,
                  in_specs=(spec,) * (n_params + n_out),
                  out_specs=(spec,) * n_out,
                  check_rep=False),
        keep_unused=True,
    )
    sharding = NamedSharding(mesh, spec)
    return fn, in_names, out_names, out_avals, sharding


def _run_spmd(nc, in_maps, time_iters=0):
    import jax
    import time

    key = id(nc)
    if key not in _RUNNER_CACHE:
        _RUNNER_CACHE[key] = _make_runner(nc)
    fn, in_names, out_names, out_avals, sharding = _RUNNER_CACHE[key]

    concat_in = [
        jax.device_put(
            np.concatenate([np.asarray(in_maps[c][n]) for c in range(C)], axis=0),
            sharding)
        for n in in_names
    ]
    concat_zero = [
        jax.device_put(np.zeros((C * a.shape[0], *a.shape[1:]), a.dtype), sharding)
        for a in out_avals
    ]
    args = concat_in + concat_zero
    out = fn(*args)
    jax.block_until_ready(out)

    if time_iters > 0:
        t0 = time.perf_counter()
        for _ in range(time_iters):
            out = fn(*args)
        jax.block_until_ready(out)
        t1 = time.perf_counter()
        LAST_STATS["wall_per_iter_ns"] = (t1 - t0) / time_iters * 1e9
        times = []
        for _ in range(time_iters):
            t0 = time.perf_counter()
            jax.block_until_ready(fn(*args))
            times.append(time.perf_counter() - t0)
        LAST_STATS["wall_min_ns"] = min(times) * 1e9

    return [
        {n: np.asarray(out[i]).reshape(C, *out_avals[i].shape)[c]
         for i, n in enumerate(out_names)}
        for c in range(C)
    ]


# ----------------------------------------------------------------------------
# Entry point
# ----------------------------------------------------------------------------

def kernel(x, mesh_edge_attr, world_edge_attr, mesh_dst, world_dst,
           W1, b1, W2, b2, gamma, beta):
    x = np.asarray(x, dtype=np.float32)
    W1 = np.asarray(W1, dtype=np.float32)
    W2 = np.asarray(W2, dtype=np.float32)
    b1 = np.asarray(b1, dtype=np.float32)
    b2 = np.asarray(b2, dtype=np.float32)
    gamma = np.asarray(gamma, dtype=np.float32)
    beta = np.asarray(beta, dtype=np.float32)

    pk = _pack(x, np.asarray(mesh_edge_attr, dtype=np.float32),
               np.asarray(world_edge_attr, dtype=np.float32),
               mesh_dst, world_dst)

    has_beta = bool(np.any(beta != 0.0))
    nc = _get_program(pk["Tm"], pk["Tw"], pk["com"], pk["cow"],
                      pk["CDTm"], pk["CDTw"], has_beta)

    w1a = np.ascontiguousarray(W1[0:D]).astype(BF16)
    w1b = np.ascontiguousarray(W1[D:2 * D]).astype(BF16)
    w1c = np.ascontiguousarray(W1[2 * D:3 * D]).astype(BF16)
    w2 = np.ascontiguousarray(W2).astype(BF16)
    b1c = np.ascontiguousarray(b1.reshape(P, 1))
    b2c = np.ascontiguousarray(b2.reshape(P, 1))
    gamma_bc = np.broadcast_to(np.tile(gamma, NB).astype(BF16),
                               (P, NB * D)).copy()

    in_maps = []
    for c in range(C):
        m = {
            "mesh_buf": pk["mesh_buf"][c],
            "world_buf": pk["world_buf"][c],
            "x_buf": pk["x_buf"][c],
            "xbf_buf": pk["xbf_buf"][c],
            "rm_buf": pk["rm_buf"][c],
            "rw_buf": pk["rw_buf"][c],
            "w1a": w1a, "w1b": w1b, "w1c": w1c, "w2": w2,
            "b1c": b1c, "b2c": b2c, "gamma_bc": gamma_bc,
        }
        if has_beta:
            m["beta_bc"] = np.broadcast_to(np.tile(beta, NB),
                                           (P, NB * D)).astype(np.float32).copy()
        in_maps.append(m)

    results = _run_spmd(nc, in_maps,
                        time_iters=int(os.environ.get("KERNEL_TIME_ITERS", "0")))

    out_stack = np.stack([results[c]["out_buf"] for c in range(C)])
    c_idx, row_idx = pk["unperm"]
    out = np.empty((N_NODES, D), dtype=np.float32)
    out[pk["order"]] = out_stack[c_idx, row_idx]
    return out


# revision 46
# speedup vs baseline: 22.0880x; 22.0880x over previous
"""Trainium2 Bass kernel for nn_NodeModel (GNN message passing).

Math (see reference):
  mesh_agg = scatter_mean(mesh_edge_attr, mesh_dst, N)
  world_agg = scatter_mean(world_edge_attr, world_dst, N)
  h = relu(concat([x, mesh_agg, world_agg]) @ W1 + b1) @ W2 + b2
  out = x + LayerNorm(h) * gamma + beta

Strategy:
  - Nodes are globally sorted by (mesh_degree, world_degree) and packed into
    784 windows of 128 nodes; windows are dealt round-robin to 8 cores so the
    per-program-slot max degrees match across cores (the per-slot edge-slot
    counts are baked into the single SPMD program).
  - Edges are packed host-side into an ELL-ish layout: window slab
    [lane(node)=partition, feat, slot] in bf16, zero padded.  On device the
    scatter-mean becomes one strided tensor_reduce (slot axis innermost) plus
    a per-node reciprocal-degree scale.
  - MLP runs feature-major on the PE: rhs operands (x^T, magg^T, wagg^T) are
    produced with DMA(xbar) transposes in bf16, weights are bf16, psum fp32.
  - LayerNorm runs node-major after a batched DMA tile-transpose; residual add
    against fp32 x.
  - All 8 cores run the same program on different data; host gathers and
    inverse-permutes the output.
"""

import os
import sys
from functools import lru_cache

import numpy as np

sys.path.insert(0, "/opt/trn_rl_repo")

import ml_dtypes

N_NODES = 100000
N_MESH = 600000
N_WORLD = 300000
D = 128
P = 128
C = 8  # cores
EPS = 1e-5
WPC = -(-N_NODES // (C * P))  # 98 windows per core
NW_TOT = C * WPC  # 784 global windows
NS = NW_TOT * P  # 100352 node slots
NB = 4  # windows per MLP batch

BF16 = ml_dtypes.bfloat16

LAST_STATS = {}


# ----------------------------------------------------------------------------
# Host-side packing
# ----------------------------------------------------------------------------

def _tiles(a):
    return np.maximum(a, 1)


def _pack(x, mesh_edge_attr, world_edge_attr, mesh_dst, world_dst):
    """Build per-core device buffers + metadata."""
    mesh_dst = np.asarray(mesh_dst).astype(np.int64)
    world_dst = np.asarray(world_dst).astype(np.int64)

    dm = np.bincount(mesh_dst, minlength=N_NODES)
    dw = np.bincount(world_dst, minlength=N_NODES)

    # node order: sorted by (mesh degree, world degree)
    order = np.lexsort((dw, dm))
    pad = NS - N_NODES
    nw_tot = NW_TOT
    wpc = WPC
    ipos = np.empty(N_NODES, dtype=np.int64)
    ipos[order] = pad + np.arange(N_NODES)
    dms = np.zeros(NS, dtype=np.int64)
    dws = np.zeros(NS, dtype=np.int64)
    dms[pad:] = dm[order]
    dws[pad:] = dw[order]

    # per-window maxima, then deal windows to (core, slot) sorted by their
    # (Tm, Tw) profile so the 8 windows sharing a baked slot count are nearly
    # identical (fat dm-boundary windows cluster instead of poisoning slots).
    wmax_m = dms.reshape(nw_tot, P).max(axis=1)
    wmax_w = dws.reshape(nw_tot, P).max(axis=1)
    wrank = np.empty(nw_tot, dtype=np.int64)
    wrank[np.lexsort((wmax_w, wmax_m))] = np.arange(nw_tot)
    win_core = wrank % C          # [nw_tot]
    win_slot = wrank // C
    Tm = _tiles(np.zeros(wpc, np.int64))
    Tw = _tiles(np.zeros(wpc, np.int64))
    np.maximum.at(Tm, win_slot, _tiles(wmax_m))
    np.maximum.at(Tw, win_slot, _tiles(wmax_w))
    # single interleaved buffer: window block = mesh planes then world planes
    coe = np.concatenate([[0], np.cumsum(P * (Tm + Tw))])  # len WPC+1
    com = coe[:-1]                  # mesh plane offset within buffer
    cow = coe[:-1] + P * Tm         # world plane offset
    CDT = int(coe[-1])

    buf = np.zeros(C * P * CDT, dtype=BF16)

    def pack_edges(attr, dst, deg, co):
        # feature-major slot planes: buf[c, d, co[s] + k*P + n] = attr[e, d]
        # for edge e with dst node at (core c, prog-slot s, lane n), edge
        # slot k within that node.
        M = dst.shape[0]
        perm = np.argsort(dst, kind="stable")
        starts = np.concatenate([[0], np.cumsum(deg)])
        dst_sorted = dst[perm]
        k = np.arange(M, dtype=np.int64) - starts[dst_sorted]
        i = ipos[dst_sorted]
        g = i // P
        n = i % P
        c = win_core[g]
        s = win_slot[g]
        base = c * (P * CDT) + co[s] + k * P + n
        attr_b = np.ascontiguousarray(attr).astype(BF16)
        d_ar = np.arange(D, dtype=np.int64) * CDT
        CH = 120000
        for lo in range(0, M, CH):
            hi = min(lo + CH, M)
            idx = base[lo:hi, None] + d_ar[None, :]
            buf[idx] = attr_b[perm[lo:hi]]

    pack_edges(mesh_edge_attr, mesh_dst, dm, com)
    pack_edges(world_edge_attr, world_dst, dw, cow)
    edge_buf = buf.reshape(C, P, CDT)

    # permuted x per core: [C, wpc*P, D]
    i = ipos[order]
    g = i // P
    p = i % P
    c = win_core[g]
    s = win_slot[g]
    row = s * P + p

    x = np.ascontiguousarray(x, dtype=np.float32)
    x_buf = np.zeros((C, wpc * P, D), dtype=np.float32)
    x_buf[c, row] = x[order]

    # reciprocal degree [C, P, wpc]  (lane-major so [128, wpc] DMAs directly)
    rm = (1.0 / np.maximum(dms, 1)).astype(np.float32)
    rw = (1.0 / np.maximum(dws, 1)).astype(np.float32)
    ga = np.arange(NS) // P
    pa = np.arange(NS) % P
    rm_buf = np.zeros((C, P, wpc), dtype=np.float32)
    rw_buf = np.zeros((C, P, wpc), dtype=np.float32)
    rm_buf[win_core[ga], pa, win_slot[ga]] = rm
    rw_buf[win_core[ga], pa, win_slot[ga]] = rw

    unperm = (c, row)  # out[order] = result[c, row]
    return dict(
        Tm=Tm, Tw=Tw, coe=coe, CDT=CDT, edge_buf=edge_buf,
        x_buf=x_buf, rm_buf=rm_buf, rw_buf=rw_buf,
        order=order, unperm=unperm, wpc=wpc,
    )


# ----------------------------------------------------------------------------
# Device program
# ----------------------------------------------------------------------------

def _build_program(Tm, Tw, coe, CDT, has_beta, wpc=WPC):
    from contextlib import ExitStack
    import concourse.bass as bass
    import concourse.tile as tile
    from concourse import bacc, mybir

    f32 = mybir.dt.float32
    bf16 = mybir.dt.bfloat16
    AF = mybir.ActivationFunctionType
    OP = mybir.AluOpType
    AX = mybir.AxisListType

    nc = bacc.Bacc("TRN2", target_bir_lowering=False, debug=False,
                   enable_asserts=False, num_devices=C)

    edge_d = nc.dram_tensor("edge_buf", [P, CDT], bf16, kind="ExternalInput").ap()
    x_d = nc.dram_tensor("x_buf", [wpc * P, D], f32, kind="ExternalInput").ap()
    rm_d = nc.dram_tensor("rm_buf", [P, wpc], f32, kind="ExternalInput").ap()
    rw_d = nc.dram_tensor("rw_buf", [P, wpc], f32, kind="ExternalInput").ap()
    w1a_d = nc.dram_tensor("w1a", [D, D], bf16, kind="ExternalInput").ap()
    w1b_d = nc.dram_tensor("w1b", [D, D], bf16, kind="ExternalInput").ap()
    w1c_d = nc.dram_tensor("w1c", [D, D], bf16, kind="ExternalInput").ap()
    w2_d = nc.dram_tensor("w2", [D, D], bf16, kind="ExternalInput").ap()
    b1_d = nc.dram_tensor("b1c", [P, 1], f32, kind="ExternalInput").ap()
    b2_d = nc.dram_tensor("b2c", [P, 1], f32, kind="ExternalInput").ap()
    gb_d = nc.dram_tensor("gamma_bc", [P, NB * D], bf16, kind="ExternalInput").ap()
    ident_d = nc.dram_tensor("ident", [P, P], bf16, kind="ExternalInput").ap()
    if has_beta:
        bb_d = nc.dram_tensor("beta_bc", [P, NB * D], f32, kind="ExternalInput").ap()
    out_d = nc.dram_tensor("out_buf", [wpc * P, D], f32, kind="ExternalOutput").ap()

    with tile.TileContext(nc) as tc, ExitStack() as ctx:
        ctx.enter_context(nc.allow_low_precision(
            reason="bf16 intermediates are intentional; DVE accumulates fp32"))
        const = ctx.enter_context(tc.tile_pool(name="const", bufs=1))
        epool = ctx.enter_context(tc.tile_pool(name="edges", bufs=3))
        xpool = ctx.enter_context(tc.tile_pool(name="xin", bufs=8))
        lpool = ctx.enter_context(tc.tile_pool(name="long", bufs=8))
        tpool = ctx.enter_context(tc.tile_pool(name="work", bufs=6))
        cpool = ctx.enter_context(tc.tile_pool(name="cwork", bufs=3))
        spool = ctx.enter_context(tc.tile_pool(name="stats", bufs=4))
        psum = ctx.enter_context(tc.tile_pool(name="psumagg", bufs=2, space="PSUM"))
        psumh = ctx.enter_context(tc.tile_pool(name="psumh", bufs=2, space="PSUM"))

        def cload(shape, dt, src, tag):
            t = const.tile(shape, dt, tag=tag)
            nc.sync.dma_start(t[:], src)
            return t

        w1a = cload([D, D], bf16, w1a_d, "w1a")
        w1b = cload([D, D], bf16, w1b_d, "w1b")
        w1c = cload([D, D], bf16, w1c_d, "w1c")
        w2 = cload([D, D], bf16, w2_d, "w2")
        b1 = cload([P, 1], f32, b1_d, "b1")
        b2 = cload([P, 1], f32, b2_d, "b2")
        gb = cload([P, NB * D], bf16, gb_d, "gb")
        if has_beta:
            bbt = cload([P, NB * D], f32, bb_d, "bbt")
        rmt = cload([P, wpc], f32, rm_d, "rmt")
        rwt = cload([P, wpc], f32, rw_d, "rwt")
        ident = cload([P, P], bf16, ident_d, "ident")
        epsc = const.tile([P, 1], f32, tag="epsc")
        nc.gpsimd.memset(epsc[:], EPS)

        batches = []
        b0 = 0
        while b0 < wpc:
            batches.append((b0, min(NB, wpc - b0)))
            b0 += NB

        state = {}

        def stage_a(bi):
            """Loads + scatter-sum on PE + mean copies + agg transposes."""
            s0, nb = batches[bi]
            col0, col1 = int(coe[s0]), int(coe[s0 + nb])

            eet = epool.tile([P, col1 - col0], bf16, tag="edges")
            nc.sync.dma_start(eet[:], edge_d[:, col0:col1])

            xt = xpool.tile([P, nb * D], f32, tag="x")
            nc.sync.dma_start(
                xt[:],
                x_d[s0 * P:(s0 + nb) * P, :].rearrange("(j p) d -> p j d", p=P),
            )

            # scatter-sum on PE: plane s_i (feature-major [d, n]) as lhsT,
            # identity rhs: psum[n, d] += plane^T (node-major, fp32).
            # aggm blocks: [0:nb]=mesh mean, [nb:2nb]=world mean,
            # [2nb:3nb]=x cast to bf16 -- one DMA transpose covers all three.
            aggm = tpool.tile([P, 3 * nb * D], bf16, tag="aggm")
            for j in range(nb):
                nc.vector.tensor_scalar(
                    aggm[:, (2 * nb + j) * D:(2 * nb + j + 1) * D],
                    xt[:, j * D:(j + 1) * D], 1.0, None, op0=OP.mult,
                )
            pm = psum.tile([P, nb * D], f32, tag="pm")
            pw = psum.tile([P, nb * D], f32, tag="pw")
            # ONE contiguous accumulation group per psum tile: start=True
            # clears accumulation state on hardware at bank granularity, so
            # only the first matmul into each tile may set it, and groups must
            # not interleave with other groups' writes.
            nm_tot = sum(int(Tm[s]) for s in range(s0, s0 + nb))
            nw_tot_ = sum(int(Tw[s]) for s in range(s0, s0 + nb))
            mi = 0
            for j in range(nb):
                s = s0 + j
                tm = int(Tm[s])
                moff = int(coe[s]) - col0
                for si in range(tm):
                    nc.tensor.matmul(
                        pm[:, j * D:(j + 1) * D],
                        eet[:, moff + si * P:moff + (si + 1) * P],
                        ident[:], start=(mi == 0), stop=(mi == nm_tot - 1),
                        skip_group_check=True,
                    )
                    mi += 1
            wi = 0
            for j in range(nb):
                s = s0 + j
                tm, tw = int(Tm[s]), int(Tw[s])
                woff = int(coe[s]) - col0 + tm * P
                for si in range(tw):
                    nc.tensor.matmul(
                        pw[:, j * D:(j + 1) * D],
                        eet[:, woff + si * P:woff + (si + 1) * P],
                        ident[:], start=(wi == 0), stop=(wi == nw_tot_ - 1),
                        skip_group_check=True,
                    )
                    wi += 1
            for j in range(nb):
                s = s0 + j
                # mean (1/deg) folds into the psum->sbuf copy scale
                nc.scalar.activation(aggm[:, j * D:(j + 1) * D],
                                     pm[:, j * D:(j + 1) * D],
                                     AF.Copy, scale=rmt[:, s:s + 1])
                nc.vector.tensor_scalar(aggm[:, (nb + j) * D:(nb + j + 1) * D],
                                        pw[:, j * D:(j + 1) * D],
                                        rwt[:, s:s + 1], None, op0=OP.mult)

            # one batched per-128-block tile transpose (node -> feat major)
            aggT = lpool.tile([P, 3 * nb, D], bf16, tag="aggT")
            nc.scalar.dma_start(aggT[:], aggm[:], transpose=True)
            state[bi] = dict(xt=xt, aggT=aggT, nb=nb)

        def stage_b(bi):
            """MLP (feature-major) + transpose back to node-major."""
            s0, nb = batches[bi]
            st = state[bi]
            h1 = psumh.tile([P, nb * D], f32, tag="h1")
            mm = st["aggT"][:, 0:nb, :].rearrange("p j d -> p (j d)")
            wm = st["aggT"][:, nb:2 * nb, :].rearrange("p j d -> p (j d)")
            xTv = st["aggT"][:, 2 * nb:3 * nb, :].rearrange("p j d -> p (j d)")
            nc.tensor.matmul(h1[:], w1a[:], xTv, start=True, stop=False)
            nc.tensor.matmul(h1[:], w1b[:], mm, start=False, stop=False)
            nc.tensor.matmul(h1[:], w1c[:], wm, start=False, stop=True)
            h1s = tpool.tile([P, nb * D], bf16, tag="h1s")
            nc.scalar.activation(h1s[:], h1[:], AF.Relu, bias=b1[:, 0:1])
            h2 = psumh.tile([P, nb * D], f32, tag="h2")
            nc.tensor.matmul(h2[:], w2[:], h1s[:], start=True, stop=True)
            yT = tpool.tile([P, nb * D], bf16, tag="yT")
            nc.scalar.activation(yT[:], h2[:], AF.Identity, bias=b2[:, 0:1])
            yn = tpool.tile([P, nb, D], bf16, tag="yn")
            nc.scalar.dma_start(yn[:], yT[:], transpose=True)
            st["yn"] = yn

        def stage_c(bi):
            """LayerNorm (node-major) + gamma/beta + residual + store."""
            s0, nb = batches[bi]
            st = state.pop(bi)
            yn, xt = st["yn"], st["xt"]

            mv = spool.tile([P, 2 * nb], f32, tag="mv")
            for j in range(nb):
                st6 = spool.tile([P, 6], f32, tag="st6")
                nc.vector.bn_stats(st6[:], yn[:, j, :])
                nc.vector.bn_aggr(mv[:, 2 * j:2 * j + 2], st6[:])
            # sd = sqrt(var + eps) ; a = 1/sd ; bb = -mu * a
            sd = spool.tile([P, nb], f32, tag="sd")
            nc.scalar.activation(sd[:], mv[:, 1::2], AF.Sqrt, bias=epsc[:, 0:1])
            av = spool.tile([P, nb], f32, tag="av")
            nc.vector.reciprocal(av[:], sd[:])
            ngm = spool.tile([P, nb], f32, tag="ngm")
            nc.vector.tensor_scalar(ngm[:], mv[:, 0::2], -1.0, None, op0=OP.mult)
            bbv = spool.tile([P, nb], f32, tag="bbv")
            nc.vector.tensor_tensor(bbv[:], ngm[:], av[:], op=OP.mult)

            tn = cpool.tile([P, nb * D], bf16, tag="tn")
            for j in range(nb):
                # t = yn * a + (-mu * a)  == (yn - mu) * rsqrt(var+eps)
                nc.scalar.activation(
                    tn[:, j * D:(j + 1) * D], yn[:, j, :], AF.Identity,
                    bias=bbv[:, j:j + 1], scale=av[:, j:j + 1],
                )
            gn = cpool.tile([P, nb * D], bf16, tag="gn")
            nc.vector.tensor_tensor(gn[:], tn[:], gb[:, :nb * D], op=OP.mult)
            on = cpool.tile([P, nb * D], f32, tag="on")
            nc.gpsimd.tensor_tensor(on[:], gn[:], xt[:], op=OP.add)
            if has_beta:
                nc.gpsimd.tensor_tensor(on[:], on[:], bbt[:, :nb * D], op=OP.add)

            nc.gpsimd.dma_start(
                out_d[s0 * P:(s0 + nb) * P, :].rearrange("(j p) d -> p j d", p=P),
                on[:],
            )

        # software-pipelined emission: A(b) | B(b-1) | C(b-2)
        nbat = len(batches)
        for b in range(nbat + 2):
            if b < nbat:
                stage_a(b)
            if 1 <= b <= nbat:
                stage_b(b - 1)
            if b >= 2:
                stage_c(b - 2)

    nc.compile()
    return nc


_PROGRAM_CACHE = {}


def _get_program(Tm, Tw, coe, CDT, has_beta, wpc=WPC):
    key = (tuple(Tm), tuple(Tw), bool(has_beta), wpc)
    if key not in _PROGRAM_CACHE:
        _PROGRAM_CACHE[key] = _build_program(Tm, Tw, coe, CDT, has_beta, wpc)
    return _PROGRAM_CACHE[key]


# ----------------------------------------------------------------------------
# SPMD runner (PJRT over axon), with optional repeat timing
# ----------------------------------------------------------------------------

_RUNNER_CACHE = {}


def _make_runner(nc):
    import jax
    from jax.sharding import Mesh, PartitionSpec, NamedSharding
    from jax.experimental.shard_map import shard_map
    from concourse import mybir
    from concourse.bass2jax import (_bass_exec_p, install_neuronx_cc_hook,
                                    partition_id_tensor)

    install_neuronx_cc_hook()

    partition_name = (nc.partition_id_tensor.name
                      if nc.partition_id_tensor else None)
    in_names, out_names, out_avals = [], [], []
    for alloc in nc.m.functions[0].allocations:
        if not isinstance(alloc, mybir.MemoryLocationSet):
            continue
        name = alloc.memorylocations[0].name
        if alloc.kind == "ExternalInput":
            if name != partition_name:
                in_names.append(name)
        elif alloc.kind == "ExternalOutput":
            out_names.append(name)
            out_avals.append(jax.core.ShapedArray(
                tuple(alloc.tensor_shape), mybir.dt.np(alloc.dtype)))
    n_params = len(in_names)
    all_names = in_names + out_names
    if partition_name is not None:
        all_names = all_names + [partition_name]

    def _body(*args):
        operands = list(args)
        if partition_name is not None:
            operands.append(partition_id_tensor())
        outs = _bass_exec_p.bind(
            *operands,
            out_avals=tuple(out_avals),
            in_names=tuple(all_names),
            out_names=tuple(out_names),
            lowering_input_output_aliases=(),
            sim_require_finite=True,
            sim_require_nnan=True,
            nc=nc,
        )
        return tuple(outs)

    devices = jax.devices()[:C]
    mesh = Mesh(np.asarray(devices), ("core",))
    spec = PartitionSpec("core")
    n_out = len(out_names)
    fn = jax.jit(
        shard_map(_body, mesh=mesh,
                  in_specs=(spec,) * (n_params + n_out),
                  out_specs=(spec,) * n_out,
                  check_rep=False),
        keep_unused=True,
    )
    sharding = NamedSharding(mesh, spec)
    return fn, in_names, out_names, out_avals, sharding


def _run_spmd(nc, in_maps, time_iters=0):
    import jax
    import time

    key = id(nc)
    if key not in _RUNNER_CACHE:
        _RUNNER_CACHE[key] = _make_runner(nc)
    fn, in_names, out_names, out_avals, sharding = _RUNNER_CACHE[key]

    concat_in = [
        jax.device_put(
            np.concatenate([np.asarray(in_maps[c][n]) for c in range(C)], axis=0),
            sharding)
        for n in in_names
    ]
    concat_zero = [
        jax.device_put(np.zeros((C * a.shape[0], *a.shape[1:]), a.dtype), sharding)
        for a in out_avals
    ]
    args = concat_in + concat_zero
    out = fn(*args)
    jax.block_until_ready(out)

    if time_iters > 0:
        t0 = time.perf_counter()
        for _ in range(time_iters):
            out = fn(*args)
        jax.block_until_ready(out)
        t1 = time.perf_counter()
        LAST_STATS["wall_per_iter_ns"] = (t1 - t0) / time_iters * 1e9
        times = []
        for _ in range(time_iters):
            t0 = time.perf_counter()
            jax.block_until_ready(fn(*args))
            times.append(time.perf_counter() - t0)
        LAST_STATS["wall_min_ns"] = min(times) * 1e9

    return [
        {n: np.asarray(out[i]).reshape(C, *out_avals[i].shape)[c]
         for i, n in enumerate(out_names)}
        for c in range(C)
    ]


# ----------------------------------------------------------------------------
# Entry point
# ----------------------------------------------------------------------------

def kernel(x, mesh_edge_attr, world_edge_attr, mesh_dst, world_dst,
           W1, b1, W2, b2, gamma, beta):
    x = np.asarray(x, dtype=np.float32)
    W1 = np.asarray(W1, dtype=np.float32)
    W2 = np.asarray(W2, dtype=np.float32)
    b1 = np.asarray(b1, dtype=np.float32)
    b2 = np.asarray(b2, dtype=np.float32)
    gamma = np.asarray(gamma, dtype=np.float32)
    beta = np.asarray(beta, dtype=np.float32)

    pk = _pack(x, np.asarray(mesh_edge_attr, dtype=np.float32),
               np.asarray(world_edge_attr, dtype=np.float32),
               mesh_dst, world_dst)

    has_beta = bool(np.any(beta != 0.0))
    nc = _get_program(pk["Tm"], pk["Tw"], pk["coe"], pk["CDT"], has_beta,
                      wpc=pk["wpc"])

    w1a = np.ascontiguousarray(W1[0:D]).astype(BF16)
    w1b = np.ascontiguousarray(W1[D:2 * D]).astype(BF16)
    w1c = np.ascontiguousarray(W1[2 * D:3 * D]).astype(BF16)
    w2 = np.ascontiguousarray(W2).astype(BF16)
    b1c = np.ascontiguousarray(b1.reshape(P, 1))
    b2c = np.ascontiguousarray(b2.reshape(P, 1))
    gamma_bc = np.broadcast_to(np.tile(gamma, NB).astype(BF16),
                               (P, NB * D)).copy()
    ident = np.eye(P, dtype=BF16)

    in_maps = []
    for c in range(C):
        m = {
            "edge_buf": pk["edge_buf"][c],
            "x_buf": pk["x_buf"][c],
            "rm_buf": pk["rm_buf"][c],
            "rw_buf": pk["rw_buf"][c],
            "w1a": w1a, "w1b": w1b, "w1c": w1c, "w2": w2,
            "b1c": b1c, "b2c": b2c, "gamma_bc": gamma_bc, "ident": ident,
        }
        if has_beta:
            m["beta_bc"] = np.broadcast_to(np.tile(beta, NB),
                                           (P, NB * D)).astype(np.float32).copy()
        in_maps.append(m)

    results = _run_spmd(nc, in_maps,
                        time_iters=int(os.environ.get("KERNEL_TIME_ITERS", "0")))

    out_stack = np.stack([results[c]["out_buf"] for c in range(C)])
    c_idx, row_idx = pk["unperm"]
    out = np.empty((N_NODES, D), dtype=np.float32)
    out[pk["order"]] = out_stack[c_idx, row_idx]
    return out
